# revision 3
# baseline (speedup 1.0000x reference)
"""ARIMA(2,1,2) residual (eps) kernel for 8 TRN2 NeuronCores.

Math
----
The reference computes, for t in [2, T) (T = len(y) - 1):

    yd[t]  = y[t+1] - y[t]
    ar[t]  = phi0*y[t] + phi1*y[t-1]
    eps[t] = yd[t] - mu - ar[t] - theta0*eps[t-1] - theta1*eps[t-2]

with eps[0] = eps[1] = 0, output out[o] = eps[o+2] for o < T-2 and
out[T-2] = out[T-1] = 0.

This is a constant-coefficient order-2 linear recurrence driven by
c[t] = y[t+1] - (1+phi0)*y[t] - phi1*y[t-1] - mu.  Its impulse response
w (w[0]=1, w[1]=-theta0, w[n]=-theta0*w[n-1]-theta1*w[n-2]) decays
geometrically (|roots| ~ 0.2 for the given coefficient scale), so eps is
numerically EXACTLY (to fp32) a short FIR of c, i.e. a short FIR of y:

    out[o] = sum_j G[j] * y[o + 3 - j],   G = conv(w[:K], [1, -(1+phi0), -phi1])

with K chosen so the dropped tail is < 1e-9.  The FIR is evaluated on
the TensorEngine as a banded matmul: lay the series time-major down the
128 partitions (columns = consecutive 128-sample blocks) and then

    OUT[:, c] = B0 @ Y[:, c] + B1 @ Y[:, c-1]

where B0/B1 are the [128,128] intra/inter-block bands of G.  Each core
processes a contiguous 1/8 slice of the output with a 1-column halo —
embarrassingly parallel, no collectives.  A tiny additive correction
fixes the first 128 outputs (recurrence warm-up) and a mask zeroes the
final two outputs.
"""

import numpy as np

import concourse.bass as bass
from concourse import mybir
from concourse.bass_utils import run_bass_kernel_spmd

NCORES = 8
N = 4194305
T = N - 1  # 4194304 outputs
S = T // NCORES  # 524288 outputs per core
C = S // 128  # 4096 output columns per core
CB = 8  # column blocks per core
BW = C // CB  # 512 columns per block (one PSUM bank)

_CACHE = {}


def _build_bass():
    f32 = mybir.dt.float32
    nc = bass.Bass()
    yt = nc.declare_dram_parameter("yt", [128, C + 1], f32, isOutput=False)
    b0t = nc.declare_dram_parameter("b0t", [128, 128], f32, isOutput=False)
    b1t = nc.declare_dram_parameter("b1t", [128, 128], f32, isOutput=False)
    corr = nc.declare_dram_parameter("corr", [128, 1], f32, isOutput=False)
    mask = nc.declare_dram_parameter("mask", [128, 1], f32, isOutput=False)
    out = nc.declare_dram_parameter("out", [128, C], f32, isOutput=True)

    from contextlib import ExitStack

    with ExitStack() as ctx:
        y_sb = ctx.enter_context(nc.sbuf_tensor("y_sb", [128, C + 1], f32))
        o_sb = ctx.enter_context(nc.sbuf_tensor("o_sb", [128, C], f32))
        b0_sb = ctx.enter_context(nc.sbuf_tensor("b0_sb", [128, 128], f32))
        b1_sb = ctx.enter_context(nc.sbuf_tensor("b1_sb", [128, 128], f32))
        corr_sb = ctx.enter_context(nc.sbuf_tensor("corr_sb", [128, 1], f32))
        mask_sb = ctx.enter_context(nc.sbuf_tensor("mask_sb", [128, 1], f32))
        psums = [
            ctx.enter_context(nc.psum_tensor("ps%d" % b, [128, BW], f32))
            for b in range(CB)
        ]
        in_sem = ctx.enter_context(nc.semaphore("in_sem"))
        mm_sem = ctx.enter_context(nc.semaphore("mm_sem"))
        cp_sem = ctx.enter_context(nc.semaphore("cp_sem"))
        out_sem = ctx.enter_context(nc.semaphore("out_sem"))
        block = ctx.enter_context(nc.Block())

        @block.sync
        def _(sync):
            sync.dma_start(out=b0_sb[:], in_=b0t[:]).then_inc(in_sem, 16)
            sync.dma_start(out=b1_sb[:], in_=b1t[:]).then_inc(in_sem, 16)
            sync.dma_start(out=corr_sb[:], in_=corr[:]).then_inc(in_sem, 16)
            sync.dma_start(out=mask_sb[:], in_=mask[:]).then_inc(in_sem, 16)
            for b in range(CB):
                lo = 0 if b == 0 else b * BW + 1
                hi = (b + 1) * BW + 1
                sync.dma_start(out=y_sb[:, lo:hi], in_=yt[:, lo:hi]).then_inc(
                    in_sem, 16
                )
            for b in range(CB):
                sync.wait_ge(cp_sem, b + 1)
                sync.dma_start(
                    out=out[:, b * BW : (b + 1) * BW],
                    in_=o_sb[:, b * BW : (b + 1) * BW],
                ).then_inc(out_sem, 16)
            sync.wait_ge(out_sem, 16 * CB)

        @block.tensor
        def _(tensor):
            for b in range(CB):
                tensor.wait_ge(in_sem, 16 * (5 + b))
                tensor.matmul(
                    psums[b][:],
                    b1_sb[:],
                    y_sb[:, b * BW : b * BW + BW],
                    start=True,
                    stop=False,
                )
                tensor.matmul(
                    psums[b][:],
                    b0_sb[:],
                    y_sb[:, b * BW + 1 : b * BW + BW + 1],
                    start=False,
                    stop=True,
                ).then_inc(mm_sem, 1)

        @block.vector
        def _(vector):
            # NOTE: back-to-back DVE ops do NOT interlock RAW through SBUF
            # (no drain between ops), so the corr/mask fixups read straight
            # from PSUM and write columns disjoint from the bulk copy.
            for b in range(CB):
                vector.wait_ge(mm_sem, b + 1)
                if b == 0:
                    vector.tensor_add(o_sb[:, 0:1], psums[b][:, 0:1], corr_sb[:])
                    ins = vector.tensor_copy(
                        o_sb[:, 1:BW], psums[b][:, 1:BW]
                    )
                elif b == CB - 1:
                    vector.tensor_mul(
                        o_sb[:, C - 1 : C], psums[b][:, BW - 1 : BW], mask_sb[:]
                    )
                    ins = vector.tensor_copy(
                        o_sb[:, b * BW : C - 1], psums[b][:, 0 : BW - 1]
                    )
                else:
                    ins = vector.tensor_copy(
                        o_sb[:, b * BW : (b + 1) * BW], psums[b][:]
                    )
                ins.then_inc(cp_sem, 1)

    return nc


def _host_prep(y, phi, theta, mu):
    """All small host-side constant prep (f64): FIR taps, band matrices,
    warm-up correction, tail mask, and the per-core input layout."""
    y = np.asarray(y, dtype=np.float32)
    p0, p1 = float(phi[0]), float(phi[1])
    t0, t1 = float(theta[0]), float(theta[1])
    m = float(mu[0])

    # Fold a nonzero mu into a constant shift of y (exact when the AR
    # stencil gain is nonzero; mu is zeros for this problem's inputs).
    if m != 0.0 and abs(p0 + p1) > 1e-12:
        y = y - np.float32(m / (-(p0 + p1)))

    # Impulse response of 1/(1 + t0 z + t1 z^2), truncated adaptively.
    wmax = 126
    w = np.zeros(wmax + 2, dtype=np.float64)
    w[0] = 1.0
    w[1] = -t0
    for n in range(2, wmax + 2):
        w[n] = -t0 * w[n - 1] - t1 * w[n - 2]
    K = wmax
    run = 0.0
    for k in range(4, wmax):
        run = abs(w[k]) + abs(w[k + 1])
        if run < 1e-10:
            K = k
            break
    w = w[:K]
    a = np.array([1.0, -(1.0 + p0), -p1], dtype=np.float64)
    G = np.convolve(w, a)  # length J = K + 2
    J = len(G)
    assert J <= 128

    # Band matrices (transposed for the matmul's lhsT operand).
    p_idx = np.arange(128)[None, :]
    q_idx = np.arange(128)[:, None]
    d0 = p_idx - q_idx
    d1 = d0 + 128
    b0t_np = np.where((d0 >= 0) & (d0 < J), G[np.clip(d0, 0, J - 1)], 0.0)
    b1t_np = np.where((d1 >= 0) & (d1 < J), G[np.clip(d1, 0, J - 1)], 0.0)
    b0t_np = np.ascontiguousarray(b0t_np, dtype=np.float32)
    b1t_np = np.ascontiguousarray(b1t_np, dtype=np.float32)

    # Warm-up correction for the first 128 outputs: true recurrence
    # minus what the FIR computes (both in f64).
    y64 = y[: 140 + J].astype(np.float64)
    eps = np.zeros(132, dtype=np.float64)  # eps[t] for t in [0, 132)
    for t in range(2, 132):
        c = y64[t + 1] - (1.0 + p0) * y64[t] - p1 * y64[t - 1] - m
        eps[t] = c - t0 * eps[t - 1] - t1 * eps[t - 2]
    y_ext = np.concatenate([np.zeros(J, dtype=np.float64), y64])
    fir0 = np.zeros(128, dtype=np.float64)
    for o in range(128):
        acc = 0.0
        for j in range(J):
            acc += G[j] * y_ext[J + o + 3 - j]
        fir0[o] = acc
    corr0 = (eps[2:130] - fir0).astype(np.float32).reshape(128, 1)

    mask7 = np.ones((128, 1), dtype=np.float32)
    mask7[126, 0] = 0.0
    mask7[127, 0] = 0.0

    # Per-core inputs: yt[q, cc] = y_ext[m*S + (cc-1)*128 + q + 3]
    # (zero-padded outside [0, N)).
    ypad = np.pad(y, (128, 160))
    zeros_c = np.zeros((128, 1), dtype=np.float32)
    ones_c = np.ones((128, 1), dtype=np.float32)
    in_maps = []
    for mcore in range(NCORES):
        flat = ypad[mcore * S + 3 : mcore * S + 3 + (C + 1) * 128]
        yt_np = np.ascontiguousarray(flat.reshape(C + 1, 128).T)
        in_maps.append(
            {
                "yt": yt_np,
                "b0t": b0t_np,
                "b1t": b1t_np,
                "corr": corr0 if mcore == 0 else zeros_c,
                "mask": mask7 if mcore == NCORES - 1 else ones_c,
            }
        )
    return in_maps


def kernel(y, phi, theta, mu):
    assert y.shape == (N,)
    if "nc" not in _CACHE:
        _CACHE["nc"] = _build_bass()
    nc = _CACHE["nc"]
    in_maps = _host_prep(y, phi, theta, mu)
    res = run_bass_kernel_spmd(nc, in_maps, list(range(NCORES)))
    out = np.empty(T, dtype=np.float32)
    for mcore in range(NCORES):
        blk = res.results[mcore]["out"]  # [128, C]
        out[mcore * S : (mcore + 1) * S] = blk.T.reshape(-1)
    return out


# revision 8
# speedup vs baseline: 1.3422x; 1.3422x over previous
"""ARIMA(2,1,2) residual (eps) kernel for 8 TRN2 NeuronCores.

Math
----
The reference computes, for t in [2, T) (T = len(y) - 1):

    yd[t]  = y[t+1] - y[t]
    ar[t]  = phi0*y[t] + phi1*y[t-1]
    eps[t] = yd[t] - mu - ar[t] - theta0*eps[t-1] - theta1*eps[t-2]

with eps[0] = eps[1] = 0, output out[o] = eps[o+2] for o < T-2 and
out[T-2] = out[T-1] = 0.

This is a constant-coefficient order-2 linear recurrence driven by
c[t] = y[t+1] - (1+phi0)*y[t] - phi1*y[t-1] - mu.  Its impulse response
w (w[0]=1, w[1]=-theta0, w[n]=-theta0*w[n-1]-theta1*w[n-2]) decays
geometrically (|roots| ~ 0.2 for the given coefficient scale), so eps is
numerically EXACTLY (to fp32) a short FIR of c, i.e. a short FIR of y:

    out[o] = sum_j G[j] * y[o + 3 - j],   G = conv(w[:K], [1, -(1+phi0), -phi1])

with K chosen so the dropped tail is < 1e-9.  The FIR is evaluated on
the TensorEngine as a banded matmul: lay the series time-major down the
128 partitions (columns = consecutive 128-sample blocks) and then

    OUT[:, c] = B0 @ Y[:, c] + B1 @ Y[:, c-1]

where B0/B1 are the [128,128] intra/inter-block bands of G.  Each core
processes a contiguous 1/8 slice of the output with a 1-column halo --
embarrassingly parallel, no collectives.  A tiny additive correction
fixes the first 128 outputs (recurrence warm-up) and a mask zeroes the
final two outputs.
"""

import numpy as np

import concourse.bass as bass
from concourse import mybir
from concourse.bass_utils import run_bass_kernel_spmd

NCORES = 8
N = 4194305
T = N - 1  # 4194304 outputs
S = T // NCORES  # 524288 outputs per core
C = S // 128  # 4096 output columns per core
CB = 8  # compute blocks per core (one PSUM bank each)
BW = C // CB  # 512 columns per block
NYC = 4  # input y DMA chunks
YCW = C // NYC  # 1024 columns per input chunk
NOC = 4  # output DMA chunks
OCW = C // NOC  # 1024 columns per output chunk

_CACHE = {}


def _build_bass():
    f32 = mybir.dt.float32
    f32r = mybir.dt.float32r
    nc = bass.Bass()
    # f32r (same fp32 bit layout from numpy's view) lets the TensorEngine
    # run at 1 cycle/row instead of fp32's 2x half-speed passes.
    yt = nc.declare_dram_parameter("yt", [128, C + 1], f32r, isOutput=False)
    # bands: cols [0:128) = B0^T, [128:256) = B1^T
    bands = nc.declare_dram_parameter("bands", [128, 256], f32r, isOutput=False)
    # cm: col 0 = corr, col 1 = mask
    cm = nc.declare_dram_parameter("cm", [128, 2], f32, isOutput=False)
    out = nc.declare_dram_parameter("out", [128, C], f32, isOutput=True)

    from contextlib import ExitStack

    with ExitStack() as ctx:
        y_sb = ctx.enter_context(nc.sbuf_tensor("y_sb", [128, C + 1], f32r))
        o_sb = ctx.enter_context(nc.sbuf_tensor("o_sb", [128, C], f32))
        k_sb = ctx.enter_context(nc.sbuf_tensor("k_sb", [128, 256], f32r))
        cm_sb = ctx.enter_context(nc.sbuf_tensor("cm_sb", [128, 2], f32))
        psums = [
            ctx.enter_context(nc.psum_tensor("ps%d" % b, [128, BW], f32))
            for b in range(CB)
        ]
        in_sem = ctx.enter_context(nc.semaphore("in_sem"))
        mm_sem = ctx.enter_context(nc.semaphore("mm_sem"))
        cp_sem = ctx.enter_context(nc.semaphore("cp_sem"))
        out_sem = ctx.enter_context(nc.semaphore("out_sem"))
        block = ctx.enter_context(nc.Block())

        b0_ap = k_sb[:, 0:128]
        b1_ap = k_sb[:, 128:256]
        corr_ap = cm_sb[:, 0:1]
        mask_ap = cm_sb[:, 1:2]

        @block.sync
        def _(sync):
            sync.dma_start(out=k_sb[:], in_=bands[:]).then_inc(in_sem, 16)
            sync.dma_start(out=cm_sb[:], in_=cm[:]).then_inc(in_sem, 16)
            for g in range(NYC):
                lo = 0 if g == 0 else g * YCW + 1
                hi = (g + 1) * YCW + 1
                sync.dma_start(out=y_sb[:, lo:hi], in_=yt[:, lo:hi]).then_inc(
                    in_sem, 16
                )
            sync.wait_ge(out_sem, 16 * NOC)

        @block.tensor
        def _(tensor):
            for b in range(CB):
                tensor.wait_ge(in_sem, 16 * (3 + b // 2))
                tensor.matmul(
                    psums[b][:],
                    b1_ap,
                    y_sb[:, b * BW : b * BW + BW],
                    start=True,
                    stop=False,
                )
                tensor.matmul(
                    psums[b][:],
                    b0_ap,
                    y_sb[:, b * BW + 1 : b * BW + BW + 1],
                    start=False,
                    stop=True,
                ).then_inc(mm_sem, 1)

        @block.vector
        def _(vector):
            # NOTE: back-to-back DVE ops do NOT interlock RAW through SBUF
            # (no drain between ops), so the corr/mask fixups read straight
            # from PSUM and write columns disjoint from the bulk copy.
            for b in range(CB):
                vector.wait_ge(mm_sem, b + 1)
                if b == 0:
                    vector.tensor_add(o_sb[:, 0:1], psums[b][:, 0:1], corr_ap)
                    ins = vector.tensor_copy(o_sb[:, 1:BW], psums[b][:, 1:BW])
                elif b == CB - 1:
                    vector.tensor_mul(
                        o_sb[:, C - 1 : C], psums[b][:, BW - 1 : BW], mask_ap
                    )
                    ins = vector.tensor_copy(
                        o_sb[:, b * BW : C - 1], psums[b][:, 0 : BW - 1]
                    )
                else:
                    ins = vector.tensor_copy(
                        o_sb[:, b * BW : (b + 1) * BW], psums[b][:]
                    )
                ins.then_inc(cp_sem, 1)

        @block.scalar
        def _(scalar):
            blocks_per_chunk = OCW // BW
            for g in range(NOC):
                scalar.wait_ge(cp_sem, blocks_per_chunk * (g + 1))
                scalar.dma_start(
                    out=out[:, g * OCW : (g + 1) * OCW],
                    in_=o_sb[:, g * OCW : (g + 1) * OCW],
                ).then_inc(out_sem, 16)

    return nc


def _host_prep(y, phi, theta, mu):
    """All small host-side constant prep (f64): FIR taps, band matrices,
    warm-up correction, tail mask, and the per-core input layout."""
    y = np.asarray(y, dtype=np.float32)
    p0, p1 = float(phi[0]), float(phi[1])
    t0, t1 = float(theta[0]), float(theta[1])
    m = float(mu[0])

    # Fold a nonzero mu into a constant shift of y (exact when the AR
    # stencil gain is nonzero; mu is zeros for this problem's inputs).
    if m != 0.0 and abs(p0 + p1) > 1e-12:
        y = y - np.float32(m / (-(p0 + p1)))

    # Impulse response of 1/(1 + t0 z + t1 z^2), truncated adaptively.
    wmax = 126
    w = np.zeros(wmax + 2, dtype=np.float64)
    w[0] = 1.0
    w[1] = -t0
    for n in range(2, wmax + 2):
        w[n] = -t0 * w[n - 1] - t1 * w[n - 2]
    K = wmax
    for k in range(4, wmax):
        if abs(w[k]) + abs(w[k + 1]) < 1e-10:
            K = k
            break
    w = w[:K]
    a = np.array([1.0, -(1.0 + p0), -p1], dtype=np.float64)
    G = np.convolve(w, a)  # length J = K + 2
    J = len(G)
    assert J <= 128

    # Band matrices (transposed for the matmul's lhsT operand).
    p_idx = np.arange(128)[None, :]
    q_idx = np.arange(128)[:, None]
    d0 = p_idx - q_idx
    d1 = d0 + 128
    b0t_np = np.where((d0 >= 0) & (d0 < J), G[np.clip(d0, 0, J - 1)], 0.0)
    b1t_np = np.where((d1 >= 0) & (d1 < J), G[np.clip(d1, 0, J - 1)], 0.0)

    # Warm-up correction for the first 128 outputs: true recurrence
    # minus what the FIR computes (both in f64).
    y64 = y[: 140 + J].astype(np.float64)
    eps = np.zeros(132, dtype=np.float64)  # eps[t] for t in [0, 132)
    for t in range(2, 132):
        c = y64[t + 1] - (1.0 + p0) * y64[t] - p1 * y64[t - 1] - m
        eps[t] = c - t0 * eps[t - 1] - t1 * eps[t - 2]
    y_ext = np.concatenate([np.zeros(J, dtype=np.float64), y64])
    fir0 = np.zeros(128, dtype=np.float64)
    for o in range(128):
        acc = 0.0
        for j in range(J):
            acc += G[j] * y_ext[J + o + 3 - j]
        fir0[o] = acc
    corr0 = eps[2:130] - fir0

    bands_np = np.zeros((128, 256), dtype=np.float32)
    bands_np[:, 0:128] = b0t_np
    bands_np[:, 128:256] = b1t_np
    cm0 = np.zeros((128, 2), dtype=np.float32)
    cm0[:, 0] = corr0
    cm0[:, 1] = 1.0
    cm_mid = cm0.copy()
    cm_mid[:, 0] = 0.0
    cm_last = cm_mid.copy()
    cm_last[126, 1] = 0.0
    cm_last[127, 1] = 0.0

    # Per-core inputs: yt[q, cc] = y_ext[m*S + (cc-1)*128 + q + 3]
    # (zero-padded outside [0, N)).
    ypad = np.pad(y, (128, 160))
    in_maps = []
    for mcore in range(NCORES):
        flat = ypad[mcore * S + 3 : mcore * S + 3 + (C + 1) * 128]
        yt_np = np.ascontiguousarray(flat.reshape(C + 1, 128).T)
        if mcore == 0:
            kn = cm0
        elif mcore == NCORES - 1:
            kn = cm_last
        else:
            kn = cm_mid
        in_maps.append({"yt": yt_np, "bands": bands_np, "cm": kn})
    return in_maps


def kernel(y, phi, theta, mu):
    assert y.shape == (N,)
    if "nc" not in _CACHE:
        _CACHE["nc"] = _build_bass()
    nc = _CACHE["nc"]
    in_maps = _host_prep(y, phi, theta, mu)
    res = run_bass_kernel_spmd(nc, in_maps, list(range(NCORES)))
    out = np.empty(T, dtype=np.float32)
    for mcore in range(NCORES):
        blk = res.results[mcore]["out"]  # [128, C]
        out[mcore * S : (mcore + 1) * S] = blk.T.reshape(-1)
    return out
